# revision 7
# baseline (speedup 1.0000x reference)
"""DiffAug (colour + translate + cutout) Bass kernel for 8 Trainium2 cores.

Data-parallel over the batch (B=8 -> 1 sample per core). Per core:

- Inputs arrive as zero-padded [320, 3840] f32 planes per tensor
  (H padded 32+32 rows, W*D padded 384+384 elems). The translation
  out[i,j] = in[i+tx-32, j+ty-32] (zero outside) then becomes a plain
  2-D window load at dynamic offset (tx, ty*12) - the zero padding
  supplies the out-of-range zeros exactly like the reference's
  pad-and-clip gather.
- Colour transform folds to per-sample affine: out = 2s*x + d with
  d = (1-2s)*mean(x) + b - 0.5 (the contrast step is identity since
  C=1). Means are computed on-chip from the unshifted window.
- The affine offset d must not leak into translation zeros, and the
  cutout rectangle must be zeroed, so each output pixel is scaled by
  F[i,j] = inb[i]*inb[j]*(1 - cut_i[i]*cut_j[j]), built on-chip from
  iota/compare row/col vectors and two rank-1 matmul outer products
  (exact 0/1 arithmetic).
"""

import sys

sys.path.insert(0, "/opt/trn_rl_repo")

import numpy as np

import concourse.bass as bass
import concourse.tile as tile
from concourse import mybir

import bass_rust
from concourse.vector_clock import ScopedClock

H = 256
W = 256
D = 12
WD = W * D            # 3072
PAD = 32
PADH = H + 2 * PAD    # 320
PADWD = WD + 2 * PAD * D  # 3840
NCORES = 8
F32 = mybir.dt.float32
I32 = mybir.dt.int32

# ---------------------------------------------------------------------------
# Workarounds for this container's walrus build, which rejects any
# instruction carrying more than one semaphore wait.
# ---------------------------------------------------------------------------

_split_counter = [0]


def _patched_drain_and_barrier(self, tick_clock, wait_clock):
    drain_inst = self.nc.sync.drain()
    wait_clock.add_sem_waits(
        drain_inst.ins, ScopedClock({None: tick_clock.global_clock})
    )
    inst = drain_inst.ins
    si = inst.sync_info
    waits = list(si.on_wait or [])
    if len(waits) > 1:
        inst.sync_info = bass_rust.SyncInfo(on_wait=[], on_update=si.on_update)
        by_name = {h.name: h for h in self.sems.allocated().values()}
        for w in waits:
            handle = by_name.get(w.ant_name)
            if handle is None:
                handle = bass_rust.SemaphoreHandle(name=w.ant_name, num=w.id)
            self.nc.sync.wait_ge(handle, w.wait_value)

    self.nc.all_engine_barrier()
    assert self.sems is not None
    popped = self.nc._tile_sem_poison_stack.pop()
    assert popped is self._sem_poison
    self.nc.clear_and_free_semaphores(list(self.sems.allocated().values()))
    self.nc.all_engine_barrier()


tile.TileContext._drain_and_barrier = _patched_drain_and_barrier


def _split_multi_waits(nc):
    """Hoist extra semaphore waits onto standalone single-wait instructions."""
    for f in nc.m.functions:
        for b in f.blocks:
            il = b.instructions
            i = 0
            while i < len(il):
                inst = il[i]
                si = getattr(inst, "sync_info", None)
                waits = list(si.on_wait) if (si is not None and si.on_wait) else []
                if len(waits) > 1:
                    inst.sync_info = bass_rust.SyncInfo(
                        on_wait=[waits[-1]], on_update=si.on_update
                    )
                    new_insts = []
                    for w in waits[:-1]:
                        ev = bass_rust.InstEventSemaphore(
                            name=f"antsplitw_{_split_counter[0]}", ins=[], outs=[]
                        )
                        _split_counter[0] += 1
                        ev.engine = inst.engine
                        ev.sync_info = bass_rust.SyncInfo(on_wait=[w], on_update=[])
                        new_insts.append(ev)
                    il[i:i] = new_insts
                    i += len(new_insts)
                i += 1


# ---------------------------------------------------------------------------
# Device program
# ---------------------------------------------------------------------------


def _build_program(split=True):
    nc = bass.Bass("TRN2", target_bir_lowering=False, debug=False)

    srcs = [
        nc.dram_tensor(n, [PADH, PADWD], F32, kind="ExternalInput")
        for n in ("img0p", "img1p", "segp")
    ]
    meta_i = nc.dram_tensor("meta_i", [1, 4], I32, kind="ExternalInput")  # tx ty ox oy
    meta_f = nc.dram_tensor("meta_f", [1, 4], F32, kind="ExternalInput")  # b0 b1 s0 s1
    out = nc.dram_tensor("out", [3, H, WD], F32, kind="ExternalOutput")

    ts = mybir.AluOpType
    AX = mybir.AxisListType

    with tile.TileContext(nc) as tc:
        with (
            tc.tile_pool(name="small", bufs=1) as small,
            tc.tile_pool(name="vecs", bufs=1) as vecs,
            tc.tile_pool(name="ftiles", bufs=1) as ftiles,
            tc.tile_pool(name="meanp", bufs=3) as meanp,
            tc.tile_pool(name="mainp", bufs=4) as mainp,
            tc.tile_pool(name="pschunk", bufs=2, space="PSUM") as pschunk,
            tc.tile_pool(name="pssmall", bufs=2, space="PSUM") as pssmall,
        ):
            # ---- scalars -------------------------------------------------
            mi = small.tile([1, 4], I32, tag="mi")
            nc.sync.dma_start(mi[:], meta_i.ap())
            mif = small.tile([1, 4], F32, tag="mif")  # tx ty ox oy as f32
            nc.vector.tensor_copy(mif[:], mi[:])
            mf = small.tile([1, 4], F32, tag="mf")
            nc.sync.dma_start(mf[:], meta_f.ap())

            tx_rv = nc.values_load(
                mi[0:1, 0:1],
                engines=[mybir.EngineType.SP],
                min_val=0,
                max_val=64,
                skip_runtime_bounds_check=True,
            )
            ty_rv = nc.values_load(
                mi[0:1, 1:2],
                engines=[mybir.EngineType.SP],
                min_val=0,
                max_val=64,
                skip_runtime_bounds_check=True,
            )

            ones = small.tile([1, 128], F32, tag="ones")
            nc.vector.memset(ones[:], 1.0)

            # ---- means over the original windows -------------------------
            # partials columns: [img0_t0, img1_t0, img0_t1, img1_t1]
            partials = small.tile([128, 4], F32, tag="partials")
            for t in range(2):
                for k in range(2):
                    mt = meanp.tile([128, WD], F32, tag="meantile")
                    r0 = PAD + t * 128
                    nc.sync.dma_start(
                        mt[:], srcs[k].ap()[r0 : r0 + 128, PAD * D : PAD * D + WD]
                    )
                    nc.vector.tensor_reduce(
                        partials[:, t * 2 + k : t * 2 + k + 1], mt[:], AX.X, ts.add
                    )
            ones_col = small.tile([128, 1], F32, tag="ones_col")
            nc.vector.memset(ones_col[:], 1.0)
            ps_sum = pssmall.tile([1, 4], F32, tag="pssum")
            nc.tensor.matmul(ps_sum[:], ones_col[:], partials[:], start=True, stop=True)
            sums_row = small.tile([1, 4], F32, tag="sums_row")
            nc.vector.tensor_copy(sums_row[:], ps_sum[:])
            # mean_k = (sums_row[k] + sums_row[2+k]) / (H*W*D)
            m01 = small.tile([1, 2], F32, tag="m01")
            nc.vector.tensor_tensor(m01[:], sums_row[:, 0:2], sums_row[:, 2:4], ts.add)
            nc.vector.tensor_scalar(m01[:], m01[:], 1.0 / float(H * W * D), None, ts.mult)

            # a_k = 2*s_k ; d_k = mean_k - a_k*mean_k + b_k - 0.5
            ad = small.tile([1, 4], F32, tag="ad")  # [a0 a1 d0 d1]
            nc.vector.tensor_scalar(ad[:, 0:2], mf[:, 2:4], 2.0, None, ts.mult)
            t01 = small.tile([1, 2], F32, tag="t01")
            nc.vector.tensor_tensor(t01[:], ad[:, 0:2], m01[:], ts.mult)
            nc.vector.tensor_tensor(ad[:, 2:4], m01[:], t01[:], ts.subtract)
            nc.vector.tensor_tensor(ad[:, 2:4], ad[:, 2:4], mf[:, 0:2], ts.add)
            nc.vector.tensor_scalar(ad[:, 2:4], ad[:, 2:4], -0.5, None, ts.add)

            # broadcast [a0 a1 d0 d1] to all 128 partitions
            ps_ad = pssmall.tile([128, 4], F32, tag="psad")
            nc.tensor.matmul(ps_ad[:], ones[:], ad[:], start=True, stop=True)
            adb = small.tile([128, 4], F32, tag="adb")
            nc.vector.tensor_copy(adb[:], ps_ad[:])

            # ---- mask vectors -------------------------------------------
            # col vectors at W*D width (j repeated D times)
            iota_j = vecs.tile([1, WD], F32, tag="iota_j")
            nc.gpsimd.iota(
                iota_j[:],
                pattern=[[1, W], [0, D]],
                base=0,
                channel_multiplier=0,
                allow_small_or_imprecise_dtypes=True,
            )
            tmpv = vecs.tile([1, WD], F32, tag="tmpv")
            tmpv2 = vecs.tile([1, WD], F32, tag="tmpv2")
            cj = vecs.tile([1, WD], F32, tag="cj")
            qj = vecs.tile([1, WD], F32, tag="qj")
            # translation-valid col: 32 <= j + ty <= 287
            nc.vector.tensor_scalar(tmpv[:], iota_j[:], mif[:, 1:2], 32.0, ts.add, ts.is_ge)
            nc.vector.tensor_scalar(tmpv2[:], iota_j[:], mif[:, 1:2], 287.0, ts.add, ts.is_le)
            nc.vector.tensor_tensor(cj[:], tmpv[:], tmpv2[:], ts.mult)
            # cutout col: oy-64 <= j <= oy+63  <=>  j+64 >= oy and j-63 <= oy
            nc.vector.tensor_scalar(tmpv[:], iota_j[:], 64.0, mif[:, 3:4], ts.add, ts.is_ge)
            nc.vector.tensor_scalar(tmpv2[:], iota_j[:], -63.0, mif[:, 3:4], ts.add, ts.is_le)
            nc.vector.tensor_tensor(qj[:], tmpv[:], tmpv2[:], ts.mult)
            nc.vector.tensor_tensor(qj[:], qj[:], cj[:], ts.mult)

            # row vectors at H width
            iota_i = small.tile([1, H], F32, tag="iota_i")
            nc.gpsimd.iota(
                iota_i[:],
                pattern=[[1, H]],
                base=0,
                channel_multiplier=0,
                allow_small_or_imprecise_dtypes=True,
            )
            tmpr = small.tile([1, H], F32, tag="tmpr")
            tmpr2 = small.tile([1, H], F32, tag="tmpr2")
            ri = small.tile([1, H], F32, tag="ri")
            sneg = small.tile([1, H], F32, tag="sneg")
            nc.vector.tensor_scalar(tmpr[:], iota_i[:], mif[:, 0:1], 32.0, ts.add, ts.is_ge)
            nc.vector.tensor_scalar(tmpr2[:], iota_i[:], mif[:, 0:1], 287.0, ts.add, ts.is_le)
            nc.vector.tensor_tensor(ri[:], tmpr[:], tmpr2[:], ts.mult)
            nc.vector.tensor_scalar(tmpr[:], iota_i[:], 64.0, mif[:, 2:3], ts.add, ts.is_ge)
            nc.vector.tensor_scalar(tmpr2[:], iota_i[:], -63.0, mif[:, 2:3], ts.add, ts.is_le)
            nc.vector.tensor_tensor(sneg[:], tmpr[:], tmpr2[:], ts.mult)
            nc.vector.tensor_tensor(sneg[:], sneg[:], ri[:], ts.mult)
            nc.vector.tensor_scalar(sneg[:], sneg[:], -1.0, None, ts.mult)

            # ---- F masks: F_t = ri_t (x) cj  -  (ri*cut)_t (x) qj ---------
            fmask = [
                ftiles.tile([128, WD], F32, tag=f"f{t}", name=f"fmask{t}")
                for t in range(2)
            ]
            NCH = 512
            for t in range(2):
                for c in range(WD // NCH):
                    psc = pschunk.tile([128, NCH], F32, tag="psc")
                    nc.tensor.matmul(
                        psc[:],
                        ri[:, t * 128 : (t + 1) * 128],
                        cj[:, c * NCH : (c + 1) * NCH],
                        start=True,
                        stop=False,
                    )
                    nc.tensor.matmul(
                        psc[:],
                        sneg[:, t * 128 : (t + 1) * 128],
                        qj[:, c * NCH : (c + 1) * NCH],
                        start=False,
                        stop=True,
                    )
                    nc.vector.tensor_copy(fmask[t][:, c * NCH : (c + 1) * NCH], psc[:])

            # ---- translated loads + apply + store ------------------------
            for t in range(2):
                row0 = tx_rv + t * 128
                col0 = ty_rv * D
                for k in range(3):
                    x = mainp.tile([128, WD], F32, tag="x")
                    nc.sync.dma_start(
                        x[:], srcs[k].ap()[bass.ds(row0, 128), bass.ds(col0, WD)]
                    )
                    if k < 2:
                        nc.vector.tensor_scalar(
                            x[:], x[:], adb[:, k : k + 1], adb[:, 2 + k : 3 + k],
                            ts.mult, ts.add,
                        )
                    nc.vector.tensor_tensor(x[:], x[:], fmask[t][:], ts.mult)
                    nc.sync.dma_start(out.ap()[k, t * 128 : (t + 1) * 128, :], x[:])

    if split:
        _split_multi_waits(nc)
    return nc


# ---------------------------------------------------------------------------
# Host side: sharding, PJRT execution, unsharding
# ---------------------------------------------------------------------------

_RUNNER = {}


def _get_runner():
    if "fn" in _RUNNER:
        return _RUNNER
    import jax
    from jax.sharding import Mesh, PartitionSpec
    from jax.experimental.shard_map import shard_map
    from concourse import bass2jax

    bass2jax.install_neuronx_cc_hook()
    nc = _build_program()
    if not nc.is_finalized():
        nc.finalize()

    partition_name = (
        nc.partition_id_tensor.name if nc.partition_id_tensor else None
    )
    in_names = []
    out_names = []
    out_avals = []
    for alloc in nc.m.functions[0].allocations:
        if not isinstance(alloc, mybir.MemoryLocationSet):
            continue
        name = alloc.memorylocations[0].name
        if alloc.kind == "ExternalInput":
            if name != partition_name:
                in_names.append(name)
        elif alloc.kind == "ExternalOutput":
            out_names.append(name)
            out_avals.append(
                jax.core.ShapedArray(
                    tuple(alloc.tensor_shape), mybir.dt.np(alloc.dtype)
                )
            )
    n_params = len(in_names)
    all_in_names = list(in_names) + list(out_names)
    if partition_name is not None:
        all_in_names.append(partition_name)
    all_in_names = tuple(all_in_names)

    def _body(*args):
        operands = list(args)
        if partition_name is not None:
            operands.append(bass2jax.partition_id_tensor())
        outs = bass2jax._bass_exec_p.bind(
            *operands,
            out_avals=tuple(out_avals),
            in_names=all_in_names,
            out_names=tuple(out_names),
            lowering_input_output_aliases=(),
            sim_require_finite=True,
            sim_require_nnan=True,
            nc=nc,
        )
        return tuple(outs)

    devices = jax.devices()[:NCORES]
    mesh = Mesh(np.asarray(devices), ("core",))
    n_outs = len(out_names)
    sharded = jax.jit(
        shard_map(
            _body,
            mesh=mesh,
            in_specs=(PartitionSpec("core"),) * (n_params + n_outs),
            out_specs=(PartitionSpec("core"),) * n_outs,
            check_rep=False,
        ),
        donate_argnums=tuple(range(n_params, n_params + n_outs)),
        keep_unused=True,
    )

    _RUNNER.update(
        fn=sharded,
        in_names=in_names,
        out_names=out_names,
        out_avals=out_avals,
        n_params=n_params,
    )
    return _RUNNER


def _shard_inputs(img0, img1, seg, bfac, sfac, cfac, tx, ty, ox, oy):
    """Concatenated per-core input arrays (axis 0 stacked over cores)."""
    del cfac  # contrast is identity for C=1
    B = img0.shape[0]
    assert B == NCORES
    pads = {n: np.zeros((B, PADH, PADWD), np.float32) for n in ("img0p", "img1p", "segp")}
    for name, arr in (("img0p", img0), ("img1p", img1), ("segp", seg)):
        pads[name][:, PAD : PAD + H, PAD * D : PAD * D + WD] = np.asarray(
            arr, dtype=np.float32
        ).reshape(B, H, WD)
    meta_i = np.stack(
        [np.asarray(v, dtype=np.int32).reshape(B) for v in (tx, ty, ox, oy)], axis=1
    ).reshape(B, 1, 4)
    bf = np.asarray(bfac, dtype=np.float32).reshape(2, B)
    sf = np.asarray(sfac, dtype=np.float32).reshape(2, B)
    meta_f = np.stack([bf[0], bf[1], sf[0], sf[1]], axis=1).reshape(B, 1, 4)
    per_core = {
        "img0p": pads["img0p"].reshape(B * PADH, PADWD),
        "img1p": pads["img1p"].reshape(B * PADH, PADWD),
        "segp": pads["segp"].reshape(B * PADH, PADWD),
        "meta_i": meta_i.reshape(B * 1, 4),
        "meta_f": meta_f.reshape(B * 1, 4),
    }
    return per_core


def kernel(**inputs):
    r = _get_runner()
    conc = _shard_inputs(**inputs)
    ins = [conc[name] for name in r["in_names"]]
    zeros = [
        np.zeros((NCORES * a.shape[0], *a.shape[1:]), a.dtype) for a in r["out_avals"]
    ]
    outs = r["fn"](*ins, *zeros)
    (out_arr,) = [np.asarray(o) for o in outs]
    # [8*3, 256, 3072] -> [3, 8, 256, 256, 12, 1]
    out_arr = out_arr.reshape(NCORES, 3, H, W, D, 1).transpose(1, 0, 2, 3, 4, 5)
    return np.ascontiguousarray(out_arr)


# revision 10
# speedup vs baseline: 1.0197x; 1.0197x over previous
"""DiffAug (colour + translate + cutout) Bass kernel for 8 Trainium2 cores.

Data-parallel over the batch (B=8 -> 1 sample per core). Per core:

- Inputs arrive as zero-padded [320, 3840] f32 planes per tensor
  (H padded 32+32 rows, W*D padded 384+384 elems). The translation
  out[i,j] = in[i+tx-32, j+ty-32] (zero outside) then becomes a plain
  2-D window load at dynamic offset (tx, ty*12) - the zero padding
  supplies the out-of-range zeros exactly like the reference's
  pad-and-clip gather.
- Colour transform folds to per-sample affine: out = 2s*x + d with
  d = (1-2s)*mean(x) + b - 0.5 (the contrast step is identity since
  C=1). Means are computed on-chip from the unshifted window.
- The affine offset d must not leak into translation zeros, and the
  cutout rectangle must be zeroed, so each output pixel is scaled by
  F[i,j] = inb[i]*inb[j]*(1 - cut_i[i]*cut_j[j]), built on-chip from
  iota/compare row/col vectors and two rank-1 matmul outer products
  (exact 0/1 arithmetic).
"""

import sys

sys.path.insert(0, "/opt/trn_rl_repo")

import numpy as np

import concourse.bass as bass
import concourse.tile as tile
from concourse import mybir

import bass_rust
from concourse.vector_clock import ScopedClock

H = 256
W = 256
D = 12
WD = W * D            # 3072
PAD = 32
PADH = H + 2 * PAD    # 320
PADWD = WD + 2 * PAD * D  # 3840
NCORES = 8
F32 = mybir.dt.float32
I32 = mybir.dt.int32

# ---------------------------------------------------------------------------
# Workarounds for this container's walrus build, which rejects any
# instruction carrying more than one semaphore wait.
# ---------------------------------------------------------------------------

_split_counter = [0]


def _patched_drain_and_barrier(self, tick_clock, wait_clock):
    drain_inst = self.nc.sync.drain()
    wait_clock.add_sem_waits(
        drain_inst.ins, ScopedClock({None: tick_clock.global_clock})
    )
    inst = drain_inst.ins
    si = inst.sync_info
    waits = list(si.on_wait or [])
    if len(waits) > 1:
        inst.sync_info = bass_rust.SyncInfo(on_wait=[], on_update=si.on_update)
        by_name = {h.name: h for h in self.sems.allocated().values()}
        for w in waits:
            handle = by_name.get(w.ant_name)
            if handle is None:
                handle = bass_rust.SemaphoreHandle(name=w.ant_name, num=w.id)
            self.nc.sync.wait_ge(handle, w.wait_value)

    self.nc.all_engine_barrier()
    assert self.sems is not None
    popped = self.nc._tile_sem_poison_stack.pop()
    assert popped is self._sem_poison
    self.nc.clear_and_free_semaphores(list(self.sems.allocated().values()))
    self.nc.all_engine_barrier()


tile.TileContext._drain_and_barrier = _patched_drain_and_barrier


def _split_multi_waits(nc):
    """Hoist extra semaphore waits onto standalone single-wait instructions."""
    for f in nc.m.functions:
        for b in f.blocks:
            il = b.instructions
            i = 0
            while i < len(il):
                inst = il[i]
                si = getattr(inst, "sync_info", None)
                waits = list(si.on_wait) if (si is not None and si.on_wait) else []
                if len(waits) > 1:
                    inst.sync_info = bass_rust.SyncInfo(
                        on_wait=[waits[-1]], on_update=si.on_update
                    )
                    new_insts = []
                    for w in waits[:-1]:
                        ev = bass_rust.InstEventSemaphore(
                            name=f"antsplitw_{_split_counter[0]}", ins=[], outs=[]
                        )
                        _split_counter[0] += 1
                        ev.engine = inst.engine
                        ev.sync_info = bass_rust.SyncInfo(on_wait=[w], on_update=[])
                        new_insts.append(ev)
                    il[i:i] = new_insts
                    i += len(new_insts)
                i += 1


# ---------------------------------------------------------------------------
# Device program
# ---------------------------------------------------------------------------


def _build_program(split=True, iters=1):
    nc = bass.Bass("TRN2", target_bir_lowering=False, debug=False)

    srcs = [
        nc.dram_tensor(n, [PADH, PADWD], F32, kind="ExternalInput")
        for n in ("img0p", "img1p", "segp")
    ]
    meta_i = nc.dram_tensor("meta_i", [1, 4], I32, kind="ExternalInput")  # tx ty ox oy
    meta_f = nc.dram_tensor("meta_f", [1, 4], F32, kind="ExternalInput")  # b0 b1 s0 s1
    out = nc.dram_tensor("out", [3, H, WD], F32, kind="ExternalOutput")

    ts = mybir.AluOpType
    AX = mybir.AxisListType

    def emit(nc, pools, it, mif, mf, off_rvs):
        small, vecs, ftiles, meanp, mainp, pschunk, pssmall = pools

        ones = small.tile([1, 128], F32, tag="ones", name=f"ones_{it}")
        nc.vector.memset(ones[:], 1.0)

        # ---- means over the original windows -------------------------
        # partials columns: [img0_t0, img1_t0, img0_t1, img1_t1]
        partials = small.tile([128, 4], F32, tag="partials", name=f"partials_{it}")
        for t in range(2):
            for k in range(2):
                mt = meanp.tile([128, WD], F32, tag="meantile", name=f"mt_{it}_{t}_{k}")
                r0 = PAD + t * 128
                nc.sync.dma_start(
                    mt[:], srcs[k].ap()[r0 : r0 + 128, PAD * D : PAD * D + WD]
                )
                nc.vector.tensor_reduce(
                    partials[:, t * 2 + k : t * 2 + k + 1], mt[:], AX.X, ts.add
                )
        ones_col = small.tile([128, 1], F32, tag="ones_col", name=f"ones_col_{it}")
        nc.vector.memset(ones_col[:], 1.0)
        ps_sum = pssmall.tile([1, 4], F32, tag="pssum", name=f"ps_sum_{it}")
        nc.tensor.matmul(ps_sum[:], ones_col[:], partials[:], start=True, stop=True)
        sums_row = small.tile([1, 4], F32, tag="sums_row", name=f"sums_row_{it}")
        nc.vector.tensor_copy(sums_row[:], ps_sum[:])
        # mean_k = (sums_row[k] + sums_row[2+k]) / (H*W*D)
        m01 = small.tile([1, 2], F32, tag="m01", name=f"m01_{it}")
        nc.vector.tensor_tensor(m01[:], sums_row[:, 0:2], sums_row[:, 2:4], ts.add)
        nc.vector.tensor_scalar(m01[:], m01[:], 1.0 / float(H * W * D), None, ts.mult)

        # a_k = 2*s_k ; d_k = mean_k - a_k*mean_k + b_k - 0.5
        ad = small.tile([1, 4], F32, tag="ad", name=f"ad_{it}")  # [a0 a1 d0 d1]
        nc.vector.tensor_scalar(ad[:, 0:2], mf[:, 2:4], 2.0, None, ts.mult)
        t01 = small.tile([1, 2], F32, tag="t01", name=f"t01_{it}")
        nc.vector.tensor_tensor(t01[:], ad[:, 0:2], m01[:], ts.mult)
        nc.vector.tensor_tensor(ad[:, 2:4], m01[:], t01[:], ts.subtract)
        nc.vector.tensor_tensor(ad[:, 2:4], ad[:, 2:4], mf[:, 0:2], ts.add)
        nc.vector.tensor_scalar(ad[:, 2:4], ad[:, 2:4], -0.5, None, ts.add)

        # broadcast [a0 a1 d0 d1] to all 128 partitions
        ps_ad = pssmall.tile([128, 4], F32, tag="psad", name=f"ps_ad_{it}")
        nc.tensor.matmul(ps_ad[:], ones[:], ad[:], start=True, stop=True)
        adb = small.tile([128, 4], F32, tag="adb", name=f"adb_{it}")
        nc.vector.tensor_copy(adb[:], ps_ad[:])

        # ---- mask vectors -------------------------------------------
        iota_j = vecs.tile([1, WD], F32, tag="iota_j", name=f"iota_j_{it}")
        nc.gpsimd.iota(
            iota_j[:],
            pattern=[[1, W], [0, D]],
            base=0,
            channel_multiplier=0,
            allow_small_or_imprecise_dtypes=True,
        )
        tmpv = vecs.tile([1, WD], F32, tag="tmpv", name=f"tmpv_{it}")
        tmpv2 = vecs.tile([1, WD], F32, tag="tmpv2", name=f"tmpv2_{it}")
        cj = vecs.tile([1, WD], F32, tag="cj", name=f"cj_{it}")
        qj = vecs.tile([1, WD], F32, tag="qj", name=f"qj_{it}")
        # translation-valid col: 32 <= j + ty <= 287
        nc.vector.tensor_scalar(tmpv[:], iota_j[:], mif[:, 1:2], 32.0, ts.add, ts.is_ge)
        nc.vector.tensor_scalar(tmpv2[:], iota_j[:], mif[:, 1:2], 287.0, ts.add, ts.is_le)
        nc.vector.tensor_tensor(cj[:], tmpv[:], tmpv2[:], ts.mult)
        # cutout col: j+64 >= oy and j-63 <= oy
        nc.vector.tensor_scalar(tmpv[:], iota_j[:], 64.0, mif[:, 3:4], ts.add, ts.is_ge)
        nc.vector.tensor_scalar(tmpv2[:], iota_j[:], -63.0, mif[:, 3:4], ts.add, ts.is_le)
        nc.vector.tensor_tensor(qj[:], tmpv[:], tmpv2[:], ts.mult)
        nc.vector.tensor_tensor(qj[:], qj[:], cj[:], ts.mult)

        # row vectors at H width
        iota_i = small.tile([1, H], F32, tag="iota_i", name=f"iota_i_{it}")
        nc.gpsimd.iota(
            iota_i[:],
            pattern=[[1, H]],
            base=0,
            channel_multiplier=0,
            allow_small_or_imprecise_dtypes=True,
        )
        tmpr = small.tile([1, H], F32, tag="tmpr", name=f"tmpr_{it}")
        tmpr2 = small.tile([1, H], F32, tag="tmpr2", name=f"tmpr2_{it}")
        ri = small.tile([1, H], F32, tag="ri", name=f"ri_{it}")
        sneg = small.tile([1, H], F32, tag="sneg", name=f"sneg_{it}")
        nc.vector.tensor_scalar(tmpr[:], iota_i[:], mif[:, 0:1], 32.0, ts.add, ts.is_ge)
        nc.vector.tensor_scalar(tmpr2[:], iota_i[:], mif[:, 0:1], 287.0, ts.add, ts.is_le)
        nc.vector.tensor_tensor(ri[:], tmpr[:], tmpr2[:], ts.mult)
        nc.vector.tensor_scalar(tmpr[:], iota_i[:], 64.0, mif[:, 2:3], ts.add, ts.is_ge)
        nc.vector.tensor_scalar(tmpr2[:], iota_i[:], -63.0, mif[:, 2:3], ts.add, ts.is_le)
        nc.vector.tensor_tensor(sneg[:], tmpr[:], tmpr2[:], ts.mult)
        nc.vector.tensor_tensor(sneg[:], sneg[:], ri[:], ts.mult)
        nc.vector.tensor_scalar(sneg[:], sneg[:], -1.0, None, ts.mult)

        # ---- F masks: F_t = ri_t (x) cj  -  (ri*cut)_t (x) qj ---------
        fmask = [
            ftiles.tile([128, WD], F32, tag=f"f{t}", name=f"fmask_{it}_{t}")
            for t in range(2)
        ]
        NCH = 512
        for t in range(2):
            for c in range(WD // NCH):
                psc = pschunk.tile([128, NCH], F32, tag="psc", name=f"psc_{it}_{t}_{c}")
                nc.tensor.matmul(
                    psc[:],
                    ri[:, t * 128 : (t + 1) * 128],
                    cj[:, c * NCH : (c + 1) * NCH],
                    start=True,
                    stop=False,
                )
                nc.tensor.matmul(
                    psc[:],
                    sneg[:, t * 128 : (t + 1) * 128],
                    qj[:, c * NCH : (c + 1) * NCH],
                    start=False,
                    stop=True,
                )
                nc.vector.tensor_copy(fmask[t][:, c * NCH : (c + 1) * NCH], psc[:])

        # ---- translated loads + apply + store ------------------------
        for t in range(2):
            for k in range(3):
                x = mainp.tile([128, WD], F32, tag="x", name=f"x_{it}_{t}_{k}")
                src_ap = bass.AP(
                    tensor=srcs[k].ap().tensor,
                    offset=off_rvs[t],
                    ap=[[PADWD, 128], [1, WD]],
                )
                nc.sync.dma_start(x[:], src_ap)
                if k < 2:
                    nc.vector.tensor_scalar(
                        x[:], x[:], adb[:, k : k + 1], adb[:, 2 + k : 3 + k],
                        ts.mult, ts.add,
                    )
                nc.vector.tensor_tensor(x[:], x[:], fmask[t][:], ts.mult)
                nc.sync.dma_start(out.ap()[k, t * 128 : (t + 1) * 128, :], x[:])

    with tile.TileContext(nc) as tc:
        with (
            tc.tile_pool(name="small", bufs=1) as small,
            tc.tile_pool(name="vecs", bufs=1) as vecs,
            tc.tile_pool(name="ftiles", bufs=1) as ftiles,
            tc.tile_pool(name="meanp", bufs=3) as meanp,
            tc.tile_pool(name="mainp", bufs=4) as mainp,
            tc.tile_pool(name="pschunk", bufs=2, space="PSUM") as pschunk,
            tc.tile_pool(name="pssmall", bufs=2, space="PSUM") as pssmall,
        ):
            pools = (small, vecs, ftiles, meanp, mainp, pschunk, pssmall)
            mi = small.tile([1, 4], I32, tag="mi")
            nc.sync.dma_start(mi[:], meta_i.ap())
            mif = small.tile([1, 4], F32, tag="mif")
            nc.vector.tensor_copy(mif[:], mi[:])
            mf = small.tile([1, 4], F32, tag="mf")
            nc.sync.dma_start(mf[:], meta_f.ap())
            tx_rv = nc.values_load(
                mi[0:1, 0:1],
                engines=[mybir.EngineType.SP],
                min_val=0,
                max_val=64,
                skip_runtime_bounds_check=True,
            )
            ty_rv = nc.values_load(
                mi[0:1, 1:2],
                engines=[mybir.EngineType.SP],
                min_val=0,
                max_val=64,
                skip_runtime_bounds_check=True,
            )
            # element offsets of the translated windows, one per row-tile,
            # snapped into concrete SP registers once and reused by every
            # dynamic DMA (avoids per-use expression lowering / register
            # exhaustion when the body is repeated for timing)
            off_rvs = [
                nc.snap(
                    (tx_rv + t * 128) * PADWD + ty_rv * D,
                    engines=bass.OrderedSet([mybir.EngineType.SP]),
                    min_val=t * 128 * PADWD,
                    max_val=(64 + t * 128) * PADWD + 64 * D,
                )
                for t in range(2)
            ]
            for it in range(iters):
                emit(nc, pools, it, mif, mf, off_rvs)

    if split:
        _split_multi_waits(nc)
    return nc


# ---------------------------------------------------------------------------
# Host side: sharding, PJRT execution, unsharding
# ---------------------------------------------------------------------------

_RUNNER = {}


def _get_runner():
    if "fn" in _RUNNER:
        return _RUNNER
    import jax
    from jax.sharding import Mesh, PartitionSpec
    from jax.experimental.shard_map import shard_map
    from concourse import bass2jax

    bass2jax.install_neuronx_cc_hook()
    nc = _build_program()
    if not nc.is_finalized():
        nc.finalize()

    partition_name = (
        nc.partition_id_tensor.name if nc.partition_id_tensor else None
    )
    in_names = []
    out_names = []
    out_avals = []
    for alloc in nc.m.functions[0].allocations:
        if not isinstance(alloc, mybir.MemoryLocationSet):
            continue
        name = alloc.memorylocations[0].name
        if alloc.kind == "ExternalInput":
            if name != partition_name:
                in_names.append(name)
        elif alloc.kind == "ExternalOutput":
            out_names.append(name)
            out_avals.append(
                jax.core.ShapedArray(
                    tuple(alloc.tensor_shape), mybir.dt.np(alloc.dtype)
                )
            )
    n_params = len(in_names)
    all_in_names = list(in_names) + list(out_names)
    if partition_name is not None:
        all_in_names.append(partition_name)
    all_in_names = tuple(all_in_names)

    def _body(*args):
        operands = list(args)
        if partition_name is not None:
            operands.append(bass2jax.partition_id_tensor())
        outs = bass2jax._bass_exec_p.bind(
            *operands,
            out_avals=tuple(out_avals),
            in_names=all_in_names,
            out_names=tuple(out_names),
            lowering_input_output_aliases=(),
            sim_require_finite=True,
            sim_require_nnan=True,
            nc=nc,
        )
        return tuple(outs)

    devices = jax.devices()[:NCORES]
    mesh = Mesh(np.asarray(devices), ("core",))
    n_outs = len(out_names)
    sharded = jax.jit(
        shard_map(
            _body,
            mesh=mesh,
            in_specs=(PartitionSpec("core"),) * (n_params + n_outs),
            out_specs=(PartitionSpec("core"),) * n_outs,
            check_rep=False,
        ),
        donate_argnums=tuple(range(n_params, n_params + n_outs)),
        keep_unused=True,
    )

    _RUNNER.update(
        fn=sharded,
        in_names=in_names,
        out_names=out_names,
        out_avals=out_avals,
        n_params=n_params,
    )
    return _RUNNER


def _shard_inputs(img0, img1, seg, bfac, sfac, cfac, tx, ty, ox, oy):
    """Concatenated per-core input arrays (axis 0 stacked over cores)."""
    del cfac  # contrast is identity for C=1
    B = img0.shape[0]
    assert B == NCORES
    pads = {n: np.zeros((B, PADH, PADWD), np.float32) for n in ("img0p", "img1p", "segp")}
    for name, arr in (("img0p", img0), ("img1p", img1), ("segp", seg)):
        pads[name][:, PAD : PAD + H, PAD * D : PAD * D + WD] = np.asarray(
            arr, dtype=np.float32
        ).reshape(B, H, WD)
    meta_i = np.stack(
        [np.asarray(v, dtype=np.int32).reshape(B) for v in (tx, ty, ox, oy)], axis=1
    ).reshape(B, 1, 4)
    bf = np.asarray(bfac, dtype=np.float32).reshape(2, B)
    sf = np.asarray(sfac, dtype=np.float32).reshape(2, B)
    meta_f = np.stack([bf[0], bf[1], sf[0], sf[1]], axis=1).reshape(B, 1, 4)
    per_core = {
        "img0p": pads["img0p"].reshape(B * PADH, PADWD),
        "img1p": pads["img1p"].reshape(B * PADH, PADWD),
        "segp": pads["segp"].reshape(B * PADH, PADWD),
        "meta_i": meta_i.reshape(B * 1, 4),
        "meta_f": meta_f.reshape(B * 1, 4),
    }
    return per_core


def kernel(**inputs):
    r = _get_runner()
    conc = _shard_inputs(**inputs)
    ins = [conc[name] for name in r["in_names"]]
    zeros = [
        np.zeros((NCORES * a.shape[0], *a.shape[1:]), a.dtype) for a in r["out_avals"]
    ]
    outs = r["fn"](*ins, *zeros)
    (out_arr,) = [np.asarray(o) for o in outs]
    # [8*3, 256, 3072] -> [3, 8, 256, 256, 12, 1]
    out_arr = out_arr.reshape(NCORES, 3, H, W, D, 1).transpose(1, 0, 2, 3, 4, 5)
    return np.ascontiguousarray(out_arr)
